# revision 14
# baseline (speedup 1.0000x reference)
"""Trainium2 Bass kernel v2 for nn_CCPM (embedding gather + 3x[conv1d ->
k-max-pool -> relu] + dense + sigmoid), data-parallel over batch on 8 cores.

Per core (B_core = B/8, chunks of BC=128 samples):
- embedding gather: ONE hardware-DGE indirect DMA per chunk (int32 global
  row ids) straight into conv column layout [F, BC*D].
- convs: direct-row matmuls: lhsT = x column-block [K+1, 128] (row 0 = ones),
  rhs = conv matrix with bias row -> PSUM [128cols, C*W] already in row
  layout (no separate transpose for the conv output).
- per-(col,c) top-k: iterated min/max extraction for the k-th order
  statistic, unsegmented prefix-scan + segment-base subtraction for
  compaction shifts, masked shifted adds for in-order compaction.
  All elementwise work is split across DVE (vector) and Pool (gpsimd).
- layer-3 selection absorbed into the dense layer (slot masks); cross-
  partition sample reduction via a tiny PE matmul; sigmoid on Act.
"""
import sys

for p in ("/opt/trn_rl_repo", "/root/.axon_site/_ro/trn_rl_repo"):
    if p not in sys.path:
        sys.path.append(p)

import numpy as np

import concourse.bacc as bacc
import concourse.bass as bass
import concourse.mybir as mybir
import concourse.tile as tile
from concourse.bass import IndirectOffsetOnAxis
from concourse.bass_utils import run_bass_kernel_spmd

F, VOCAB, D = 23, 100000, 64
B_FULL = 16384
NCORES = 8
B_CORE = B_FULL // NCORES
BC = 128
NBLK = BC * D // 128          # 64 column blocks per chunk
ETA = 2.0 ** -96
BIG = 1e30
f32 = mybir.dt.float32
alu = mybir.AluOpType

# NBLK split point: blocks [0, A) on DVE, [A, NBLK) on Pool
SPLIT = {1: 36, 2: 36, 3: 36}


def conv_matrix(w, W_in, kw, C_in, C_out, W_out):
    """K[(w_in, c_in) w-major rows, (c_out, w_out) c-major cols]."""
    K = np.zeros((W_in * C_in, C_out * W_out), np.float32)
    for i in range(W_out):
        for j in range(kw):
            wsrc = i + j - (kw - 1)
            if 0 <= wsrc < W_in:
                for ci in range(C_in):
                    for co in range(C_out):
                        K[wsrc * C_in + ci, co * W_out + i] = w[j, ci, co]
    return K


def conv_matrix_cmajor_rows(w, W_in, kw, C_in, C_out, W_out):
    K = conv_matrix(w, W_in, kw, C_in, C_out, W_out)
    K = K.reshape(W_in, C_in, C_out * W_out).transpose(1, 0, 2)
    return np.ascontiguousarray(K.reshape(C_in * W_in, C_out * W_out))


def bcast_w(ap_3, W):
    """[128, a, c] AP -> [128, a, c, W] with stride-0 W axis."""
    return bass.AP(ap_3.tensor, ap_3.offset,
                   [list(p) for p in ap_3.ap] + [[0, W]])


def bcast_mid(ap_2, n, m):
    """[128, W] AP -> [128, n, m, W] broadcast over middle axes."""
    a = ap_2.ap
    return bass.AP(ap_2.tensor, ap_2.offset,
                   [list(a[0]), [0, n], [0, m], list(a[1])])


def engines(nc, layer):
    return [(nc.vector, 0, NBLK)]


def csplit(layer):
    """compaction block split: DVE [0, A), Pool [A, NBLK)."""
    return SPLIT[layer]


def build_program(b_core):
    nchunks = b_core // BC
    nc = bacc.Bacc(None, target_bir_lowering=False, debug=False)

    tab_d = nc.dram_tensor("tab", [F * VOCAB, D], f32, kind="ExternalInput")
    gidx_d = nc.dram_tensor("gidx", [nchunks, 128, F], mybir.dt.int32,
                            kind="ExternalInput")
    k1_d = nc.dram_tensor("k1e", [24, 112], f32, kind="ExternalInput")
    k2_d = nc.dram_tensor("k2e", [81, 96], f32, kind="ExternalInput")
    k3_d = nc.dram_tensor("k3e", [29, 36], f32, kind="ExternalInput")
    wdt_d = nc.dram_tensor("wdtx", [128, 3, 256], f32, kind="ExternalInput")
    bd_d = nc.dram_tensor("bde", [128, 1], f32, kind="ExternalInput")
    id_d = nc.dram_tensor("ident", [128, 128], f32, kind="ExternalInput")
    ineg_d = nc.dram_tensor("ioneg", [128, 28], f32, kind="ExternalInput")
    on2_d = nc.dram_tensor("ones2", [128, 2], f32, kind="ExternalInput")
    onr_d = nc.dram_tensor("onesrow", [1, BC * D], f32, kind="ExternalInput")
    import os as _os
    dbg = _os.environ.get("V2DBG") == "1"
    if dbg:
        zd1 = nc.dram_tensor("zdbg1", [128, NBLK, 80], f32, kind="ExternalOutput")
        zd2 = nc.dram_tensor("zdbg2", [128, NBLK, 28], f32, kind="ExternalOutput")
        rd1 = nc.dram_tensor("rdbg1", [128, NBLK, 112], f32, kind="ExternalOutput")
        dd = nc.dram_tensor("ddbg", [128, NBLK], f32, kind="ExternalOutput")
        wsd = nc.dram_tensor("wsdbg", [128, NBLK, 36], f32, kind="ExternalOutput")
        qd = nc.dram_tensor("qdbg", [128, NBLK, 36], f32, kind="ExternalOutput")
        phd = nc.dram_tensor("phdbg", [128, NBLK, 36], f32, kind="ExternalOutput")
        rd3 = nc.dram_tensor("rdbg3", [128, NBLK, 36], f32, kind="ExternalOutput")
    else:
        zd1 = zd2 = rd1 = dd = wsd = qd = phd = rd3 = None
    out_d = nc.dram_tensor("out", [b_core, 1], f32, kind="ExternalOutput")

    with tile.TileContext(nc) as tc:
        with tc.tile_pool(name="glob", bufs=1) as gp:
            k1e = gp.tile([24, 112], f32)
            nc.sync.dma_start(k1e[:], k1_d[:])
            k2e = gp.tile([81, 96], f32)
            nc.sync.dma_start(k2e[:], k2_d[:])
            k3e_t = gp.tile([93, 36], f32)
            nc.sync.dma_start(k3e_t[64:93], k3_d[:])
            k3e = k3e_t[64:93]
            wdt = gp.tile([128, 3, 256], f32)
            nc.sync.dma_start(wdt[:], wdt_d[:])
            bde = gp.tile([128, 1], f32)
            nc.sync.dma_start(bde[:], bd_d[:])
            ident = gp.tile([128, 128], f32)
            nc.sync.dma_start(ident[:], id_d[:])
            ioneg = gp.tile([128, 28], f32)
            nc.sync.dma_start(ioneg[:], ineg_d[:])
            ones2 = gp.tile([128, 2], f32)
            nc.sync.dma_start(ones2[:], on2_d[:])

            # big static arenas (shared across chunks)
            xcol = gp.tile([24, BC * D], f32)      # L1 gather dst, row 0=ones
            zcol = gp.tile([93, BC * D], f32)      # L2/L3 conv input, row 0=ones
            t_r = gp.tile([128, NBLK, 112], f32)   # conv outputs (row layout)
            t_w = gp.tile([128, NBLK, 112], f32)   # work / incl / shift-tilde
            t_e = gp.tile([128, NBLK, 112], f32)   # eqz / drop / wsel
            t_z = gp.tile([128, NBLK, 80], f32)    # compacted z (L1: 80, L2: 28)
            t_t = gp.tile([128, NBLK, 96], f32)    # compact tmp / q3 / scratch
            t_m = gp.tile([128, NBLK, 4], f32)     # per-group reduce result
            t_sp = gp.tile([128, NBLK * 4 + 128], f32)   # segment bases
            t_dots = gp.tile([128, NBLK], f32)
            gidx = gp.tile([128, F], mybir.dt.int32)
            erow = gp.tile([128, F, D], f32)

            nc.sync.dma_start(xcol[23:24, :], onr_d[:])
            nc.sync.dma_start(zcol[80:81, :], onr_d[:])
            nc.sync.dma_start(zcol[92:93, :], onr_d[:])

            for ch in range(nchunks):
                build_chunk(nc, tc, ch, tab_d, gidx_d, onr_d, out_d,
                            k1e, k2e, k3e, wdt, bde, ident, ioneg, ones2,
                            xcol, zcol, t_r, t_w, t_e, t_z, t_t, t_m, t_sp,
                            t_dots, gidx, erow,
                            dbg=(zd1, zd2, rd1, dd, wsd, qd, phd, rd3) if (dbg and ch == 0) else None)
    nc.compile()
    return nc


def packed(t, cw):
    """[128, NBLK, cw] packed (contiguous) view of a flat work tile."""
    return t[:].rearrange("p a m -> p (a m)")[:, 0:NBLK * cw].rearrange(
        "p (a cw) -> p a cw", cw=cw)


def conv_rows(nc, tc, ch, tag, xarena, K, kmat, M, t_r, batch=4):
    """Direct-row conv: out row-block jb = x[:, jb*128:+128].T @ kmat.
    Writes packed(t_r, M)[:, jb, :]."""
    rv = packed(t_r, M)
    with tc.tile_pool(name=f"cps{tag}_{ch}", bufs=2,
                      space=bass.MemorySpace.PSUM) as psum:
        for g in range(NBLK // batch):
            ps = psum.tile([128, batch * M], f32, name=f"ps{tag}",
                           tag=f"ps{tag}")
            for q in range(batch):
                jb = g * batch + q
                nc.tensor.matmul(ps[:, q * M:(q + 1) * M],
                                 xarena[0:K, jb * 128:(jb + 1) * 128],
                                 kmat, start=True, stop=True)
            nc.scalar.activation(
                rv[:, g * batch:(g + 1) * batch, :].rearrange(
                    "p a m -> p (a m)"),
                ps[:], mybir.ActivationFunctionType.Copy, bias=0.0)


def transpose_cols(nc, tc, ch, tag, t_z, zw, zarena, row0, batch=4):
    """z row blocks [128, zw] -> column layout into zarena rows row0..row0+zw."""
    zv = packed(t_z, zw)
    with tc.tile_pool(name=f"tps{tag}_{ch}", bufs=2,
                      space=bass.MemorySpace.PSUM) as psum:
        ident = transpose_cols.ident
        for g in range(NBLK // batch):
            ps = psum.tile([zw, batch * 128], f32, name=f"tp{tag}",
                           tag=f"tp{tag}")
            for q in range(batch):
                jb = g * batch + q
                nc.tensor.transpose(ps[:, q * 128:(q + 1) * 128],
                                    zv[:, jb, :], ident[:])
            nc.scalar.activation(
                zarena[row0:row0 + zw,
                       g * batch * 128:(g + 1) * batch * 128],
                ps[:], mybir.ActivationFunctionType.Copy, bias=0.0)


def ereduce(eng, is_dve, m, wk, W, C, op, scr_flat):
    """Per-group reduce along W into m [128, nb, C]. DVE: native X-reduce;
    Pool: pairwise fold tree (GpSimd has no free-axis tensor_reduce)."""
    if is_dve:
        eng.tensor_reduce(m, wk, axis=mybir.AxisListType.X, op=op)
        return
    w1 = (W + 1) // 2
    w2 = (w1 + 1) // 2
    regA = scr_flat[:, :, 0:C * w1].rearrange("p a (c w) -> p a c w", c=C)
    regB = scr_flat[:, :, C * w1:C * (w1 + w2)].rearrange(
        "p a (c w) -> p a c w", c=C)
    cur, width, use_a = wk, W, True
    while width > 1:
        half, rem = width // 2, width % 2
        dst = regA if use_a else regB
        eng.tensor_tensor(dst[:, :, :, 0:half], cur[:, :, :, 0:half],
                          cur[:, :, :, half:2 * half], op=op)
        if rem:
            eng.tensor_copy(dst[:, :, :, half:half + 1],
                            cur[:, :, :, 2 * half:2 * half + 1])
        width = half + rem
        cur, use_a = dst, not use_a
    eng.tensor_copy(m, cur[:, :, :, 0:1].rearrange("p a c w -> p a (c w)"))


def packed_off(t, off, cw):
    """[128, NBLK, cw] packed view of flat tile starting at free offset."""
    return t[:].rearrange("p a m -> p (a m)")[:, off:off + NBLK * cw].rearrange(
        "p (a cw) -> p a cw", cw=cw)


def bcast_g(ap_2, G):
    """[128, W] AP -> [128, G, W] broadcast over middle axis."""
    a = ap_2.ap
    return bass.AP(ap_2.tensor, ap_2.offset, [list(a[0]), [0, G], list(a[1])])


def bcast_w3(ap_2, W):
    """[128, G] AP -> [128, G, W] with stride-0 W axis."""
    return bass.AP(ap_2.tensor, ap_2.offset,
                   [list(p) for p in ap_2.ap] + [[0, W]])


def g3(pv, a0, a1, C):
    """packed [128, NBLK, C*W] view -> [128, (nb C), W] 3D slice."""
    return pv[:, a0:a1].rearrange("p a (c w) -> p (a c) w", c=C)


def ereduce3(eng, is_dve, m, wk, W, scrA, scrB):
    """reduce along W: wk [128, G, W] -> m [128, G]."""
    if is_dve:
        eng.tensor_reduce(m, wk, axis=mybir.AxisListType.X, op=ereduce3.op)
        return
    op = ereduce3.op
    cur, width, use_a = wk, W, True
    while width > 1:
        half, rem = width // 2, width % 2
        dst = scrA if use_a else scrB
        eng.tensor_tensor(dst[:, :, 0:half], cur[:, :, 0:half],
                          cur[:, :, half:2 * half], op=op)
        if rem:
            eng.tensor_copy(dst[:, :, half:half + 1],
                            cur[:, :, 2 * half:2 * half + 1])
        width = half + rem
        cur, use_a = dst, not use_a
    eng.tensor_copy(m, cur[:, :, 0])


def topk_stage(nc, layer, W, C, k, largest, t_r, t_w, t_e, t_z, t_t, t_m,
               t_sp, ioneg, uniquify):
    """Full top-k + compaction for one layer, split across DVE and Pool.
    All stt ops use 3D [128, G=(nb*C), W] views (walrus requires <=3D)."""
    nrounds = k if largest else W - k
    CW = C * W
    rv, wv, ev = packed(t_r, CW), packed(t_w, CW), packed(t_e, CW)
    zv = packed(t_z, C * k)
    mv = packed(t_m, 4)
    w1 = (W + 1) // 2
    w2 = (w1 + 1) // 2
    sA = packed_off(t_t, 0, C * w1)
    sB = packed_off(t_t, NBLK * C * w1, C * w2)
    for eng, a0, a1 in engines(nc, layer):
        nb = a1 - a0
        r = g3(rv, a0, a1, C)
        wk = g3(wv, a0, a1, C)
        if uniquify:
            eng.scalar_tensor_tensor(wk, r, 0.0,
                                     bcast_g(ioneg[:, 0:W], nb * C),
                                     op0=alu.is_equal, op1=alu.mult)
            eng.tensor_tensor(r, r, wk, op=alu.add)

    # extraction works on a copy; t_r stays pristine for drop + compaction
    nc.scalar.copy(wv.rearrange("p a cw -> p (a cw)"),
                   rv.rearrange("p a cw -> p (a cw)"))

    red_op = alu.max if largest else alu.min
    kill = -BIG if largest else BIG
    for ei, (eng, a0, a1) in enumerate(engines(nc, layer)):
        nb = a1 - a0
        G = nb * C
        r = g3(rv, a0, a1, C)
        wk = g3(wv, a0, a1, C)
        eq = g3(ev, a0, a1, C)
        m = mv[:, a0:a1].rearrange("p a c -> p (a c)")
        mb = bcast_w3(m, W)
        ereduce3.op = red_op
        for t in range(nrounds):
            ereduce3(eng, ei == 0, m, wk, W,
                     g3(sA, a0, a1, C), g3(sB, a0, a1, C))
            if t < nrounds - 1:
                eng.tensor_tensor(eq, wk, mb, op=alu.is_equal)
                eng.scalar_tensor_tensor(wk, eq, kill, wk,
                                         op0=alu.mult, op1=alu.add)
        # drop mask vs tau (= last extracted value)
        dop = alu.is_lt if largest else alu.is_le
        eng.tensor_tensor(eq, r, mb, op=dop)
        # compaction shifts: unsegmented cumsum + segment-base subtract
        eng.tensor_tensor_scan(
            wv[:, a0:a1].rearrange("p a cw -> p (a cw)"),
            ev[:, a0:a1].rearrange("p a cw -> p (a cw)"),
            ev[:, a0:a1].rearrange("p a cw -> p (a cw)"),
            0.0, op0=alu.add, op1=alu.bypass)
        incl = wk
        sp2 = t_sp[:, a0 * C:a1 * C]
        eng.memset(sp2[:, 0:1], 0.0)
        eng.tensor_copy(sp2[:, 1:G], incl[:, 0:G - 1, W - 1])
        eng.tensor_tensor(incl, incl, bcast_w3(sp2, W), op=alu.subtract)
        # shift-tilde in place: st = 99*drop + (incl - segbase)
        eng.scalar_tensor_tensor(incl, eq, 99.0, incl,
                                 op0=alu.mult, op1=alu.add)
    # in-order compaction: z[t] = r[t+s] where st[t+s] == s
    # DVE blocks [0, A): fused stt form; Pool blocks [A, NBLK): ts+tt form.
    # Block-grouped so PE transposes / Act relu can start on early groups
    # while later groups still compact.
    A = csplit(layer)
    stv, srv = wv, rv
    ttv = packed_off(t_t, 0, C * k)
    mkv = packed_off(t_e, 0, C * k)   # pool mask scratch (drop mask is dead)
    dve_groups = [(g, min(g + 12, A)) for g in range(0, A, 12)]
    for g0, g1 in dve_groups:
        for s in range(W - k + 1):
            st3 = g3(stv, g0, g1, C)
            sr3 = g3(srv, g0, g1, C)
            z = g3(zv, g0, g1, C)
            tt = g3(ttv, g0, g1, C)
            if s == 0:
                nc.vector.scalar_tensor_tensor(z, st3[:, :, 0:k], 0.0,
                                               sr3[:, :, 0:k],
                                               op0=alu.is_equal, op1=alu.mult)
            else:
                nc.vector.scalar_tensor_tensor(tt, st3[:, :, s:s + k],
                                               float(s), sr3[:, :, s:s + k],
                                               op0=alu.is_equal, op1=alu.mult)
                nc.vector.tensor_tensor(z, z, tt, op=alu.add)
    pool_groups = [(g, min(g + 14, NBLK)) for g in range(A, NBLK, 14)]
    for g0, g1 in pool_groups:
        for s in range(W - k + 1):
            st3 = g3(stv, g0, g1, C)
            sr3 = g3(srv, g0, g1, C)
            z = g3(zv, g0, g1, C)
            tt = g3(ttv, g0, g1, C)
            mk = g3(mkv, g0, g1, C)
            nc.gpsimd.tensor_scalar(mk, st3[:, :, s:s + k], float(s), None,
                                    op0=alu.is_equal)
            if s == 0:
                nc.gpsimd.tensor_tensor(z, mk, sr3[:, :, 0:k], op=alu.mult)
            else:
                nc.gpsimd.tensor_tensor(tt, mk, sr3[:, :, s:s + k],
                                        op=alu.mult)
                nc.gpsimd.tensor_tensor(z, z, tt, op=alu.add)


def build_chunk(nc, tc, ch, tab_d, gidx_d, onr_d, out_d, k1e, k2e, k3e,
                wdt, bde, ident, ioneg, ones2, xcol, zcol, t_r, t_w, t_e,
                t_z, t_t, t_m, t_sp, t_dots, gidx, erow, dbg=None):
    transpose_cols.ident = ident

    # ---------------- gather (per-feature, one offset per partition) ------
    nc.sync.dma_start(gidx[:], gidx_d[ch])
    for f in range(F):
        nc.gpsimd.indirect_dma_start(
            out=erow[:, f, :],
            out_offset=None,
            in_=tab_d[:],
            in_offset=IndirectOffsetOnAxis(ap=gidx[:, f:f + 1], axis=0))
    # transpose to column layout xcol[f, (h b)]
    with tc.tile_pool(name=f"eps_{ch}", bufs=2,
                      space=bass.MemorySpace.PSUM) as epsum:
        xv = xcol[0:F, :].rearrange("f (h b) -> f h b", h=D)
        for g in range(D // 4):
            pe = epsum.tile([F, 4 * 128], f32, name="pte", tag="pte")
            for q in range(4):
                h = g * 4 + q
                nc.tensor.transpose(pe[:, q * 128:(q + 1) * 128],
                                    erow[:, :, h], ident[:])
            nc.scalar.activation(
                xv[:, g * 4:(g + 1) * 4, :].rearrange("f h b -> f (h b)"),
                pe[:], mybir.ActivationFunctionType.Copy, bias=0.0)

    # ---------------- layer 1 ----------------
    conv_rows(nc, tc, ch, "1", xcol, 24, k1e[:], 112, t_r, batch=4)
    topk_stage(nc, 1, 28, 4, 20, False, t_r, t_w, t_e, t_z, t_t, t_m,
               t_sp, ioneg, uniquify=False)
    if dbg is not None:
        nc.sync.dma_start(dbg[2][:], packed(t_r, 112))
    zv1g = packed(t_z, 80)
    for g in range(0, NBLK, 16):
        seg = zv1g[:, g:g + 16].rearrange("p a cw -> p (a cw)")
        nc.scalar.activation(seg, seg, mybir.ActivationFunctionType.Relu)
    if dbg is not None:
        nc.sync.dma_start(dbg[0][:], packed(t_z, 80))
    transpose_cols(nc, tc, ch, "1", t_z, 80, zcol, 0, batch=4)

    # ---------------- layer 2 ----------------
    conv_rows(nc, tc, ch, "2", zcol, 81, k2e[:], 96, t_r, batch=4)
    topk_stage(nc, 2, 24, 4, 7, True, t_r, t_w, t_e, t_z, t_t, t_m,
               t_sp, ioneg, uniquify=True)
    zv2g = packed(t_z, 28)
    for g in range(0, NBLK, 16):
        seg = zv2g[:, g:g + 16].rearrange("p a cw -> p (a cw)")
        nc.scalar.activation(seg, seg, mybir.ActivationFunctionType.Relu)
    if dbg is not None:
        nc.sync.dma_start(dbg[1][:], packed(t_z, 28))
    transpose_cols(nc, tc, ch, "2", t_z, 28, zcol, 64, batch=4)

    # ---------------- layer 3 + dense ----------------
    conv_rows(nc, tc, ch, "3", zcol[64:93], 29, k3e, 36, t_r, batch=8)
    # L3 transposes clobbered L2's ones row (80); restore AFTER conv3 read it
    nc.sync.dma_start(zcol[80:81, :], onr_d[:])
    if dbg is not None:
        nc.sync.dma_start(dbg[7][:], packed(t_r, 36))
    W, C, k = 9, 4, 3
    rv, wv, ev = packed(t_r, 36), packed(t_w, 36), packed(t_e, 36)
    qv = packed(t_t, 36)
    sv = packed(t_z, 36)  # scratch (z2k already consumed)
    mv = packed(t_m, 4)
    sA = packed_off(t_z, NBLK * 36, 4 * 5)   # pool reduce scratch in t_z tail
    sB = packed_off(t_z, NBLK * 36 + NBLK * 20, 4 * 3)
    for eng, a0, a1 in engines(nc, 3):
        nb = a1 - a0
        r = g3(rv, a0, a1, C)
        wk = g3(wv, a0, a1, C)
        eng.scalar_tensor_tensor(wk, r, 0.0, bcast_g(ioneg[:, 0:W], nb * C),
                                 op0=alu.is_equal, op1=alu.mult)
        eng.tensor_tensor(r, r, wk, op=alu.add)
    nc.scalar.copy(wv.rearrange("p a cw -> p (a cw)"),
                   rv.rearrange("p a cw -> p (a cw)"))
    for ei, (eng, a0, a1) in enumerate(engines(nc, 3)):
        nb = a1 - a0
        G = nb * C
        r = g3(rv, a0, a1, C)
        wk = g3(wv, a0, a1, C)
        eq = g3(ev, a0, a1, C)
        m = mv[:, a0:a1].rearrange("p a c -> p (a c)")
        mb = bcast_w3(m, W)
        ereduce3.op = alu.max
        for t in range(k):
            ereduce3(eng, ei == 0, m, wk, W,
                     g3(sA, a0, a1, C), g3(sB, a0, a1, C))
            if t < k - 1:
                eng.tensor_tensor(eq, wk, mb, op=alu.is_equal)
                eng.scalar_tensor_tensor(wk, eq, -BIG, wk,
                                         op0=alu.mult, op1=alu.add)
        eng.tensor_tensor(eq, r, mb, op=alu.is_ge)  # sel
        eng.tensor_tensor_scan(
            wv[:, a0:a1].rearrange("p a cw -> p (a cw)"),
            ev[:, a0:a1].rearrange("p a cw -> p (a cw)"),
            ev[:, a0:a1].rearrange("p a cw -> p (a cw)"),
            0.0, op0=alu.add, op1=alu.bypass)
        incl = wk
        sp2 = t_sp[:, a0 * C:a1 * C]
        eng.memset(sp2[:, 0:1], 0.0)
        eng.tensor_copy(sp2[:, 1:G], incl[:, 0:G - 1, W - 1])
        eng.tensor_tensor(incl, incl, bcast_w3(sp2, W), op=alu.subtract)
        # phat = incl - 101*sel: selected -> {-100,-99,-98}, dropped >= 0
        eng.scalar_tensor_tensor(incl, eq, -101.0, incl,
                                 op0=alu.mult, op1=alu.add)
    # q3 = relu(r3) on Act
    nc.scalar.activation(qv.rearrange("p a cw -> p (a cw)"),
                         rv.rearrange("p a cw -> p (a cw)"),
                         mybir.ActivationFunctionType.Relu)
    # wsel/dots: DVE blocks [0, A3), Pool blocks [A3, NBLK) in legal forms
    A3 = 40
    for part, (a0, a1) in enumerate(((0, A3), (A3, NBLK))):
        nb = a1 - a0
        G = nb * C
        phat = g3(wv, a0, a1, C)
        wsel = g3(ev, a0, a1, C)
        scr = g3(sv, a0, a1, C)
        q3 = g3(qv, a0, a1, C)
        wtbs = []
        for t in range(k):
            wtbs.append(bass.AP(
                wdt.tensor, wdt[:, t, a0 * C:a1 * C].offset,
                [list(wdt[:].ap[0]), [1, G], [0, W]]))
        if part == 0:
            for t in range(k):
                if t == 0:
                    nc.vector.scalar_tensor_tensor(
                        wsel, phat, float(t - 100), wtbs[t],
                        op0=alu.is_equal, op1=alu.mult)
                else:
                    nc.vector.scalar_tensor_tensor(
                        scr, phat, float(t - 100), wtbs[t],
                        op0=alu.is_equal, op1=alu.mult)
                    nc.vector.tensor_tensor(wsel, wsel, scr, op=alu.add)
            nc.vector.tensor_tensor(q3, q3, wsel, op=alu.mult)
            m = mv[:, a0:a1].rearrange("p a c -> p (a c)")
            nc.vector.tensor_reduce(m, q3, axis=mybir.AxisListType.X,
                                    op=alu.add)
            m4 = mv[:, a0:a1]
            s2 = t_sp[:, 256 + a0 * 2:256 + a0 * 2 + nb * 2].rearrange(
                "p (a c) -> p a c", c=2)
            nc.vector.tensor_tensor(s2, m4[:, :, 0:2], m4[:, :, 2:4],
                                    op=alu.add)
            nc.vector.tensor_tensor(t_dots[:, a0:a1], s2[:, :, 0],
                                    s2[:, :, 1], op=alu.add)
        else:
            for t in range(k):
                if t == 0:
                    nc.gpsimd.tensor_scalar(wsel, phat, float(t - 100), None,
                                            op0=alu.is_equal)
                    nc.gpsimd.tensor_tensor(wsel, wsel, wtbs[t], op=alu.mult)
                else:
                    nc.gpsimd.tensor_scalar(scr, phat, float(t - 100), None,
                                            op0=alu.is_equal)
                    nc.gpsimd.tensor_tensor(scr, scr, wtbs[t], op=alu.mult)
                    nc.gpsimd.tensor_tensor(wsel, wsel, scr, op=alu.add)
            nc.gpsimd.tensor_tensor(q3, q3, wsel, op=alu.mult)
            # add-fold tree (pool-legal): 36 -> 1 per group via halving on W
            m = mv[:, a0:a1].rearrange("p a c -> p (a c)")
            ereduce3.op = alu.add
            ereduce3(nc.gpsimd, False, m, q3, W,
                     g3(sA, a0, a1, C), g3(sB, a0, a1, C))
            m4 = mv[:, a0:a1]
            s2 = t_sp[:, 256 + a0 * 2:256 + a0 * 2 + nb * 2].rearrange(
                "p (a c) -> p a c", c=2)
            nc.gpsimd.tensor_tensor(s2, m4[:, :, 0:2], m4[:, :, 2:4],
                                    op=alu.add)
            nc.gpsimd.tensor_tensor(t_dots[:, a0:a1], s2[:, :, 0],
                                    s2[:, :, 1], op=alu.add)
    # per-sample logit: partition p = b -> reduce over h (free axis)
    with tc.tile_pool(name=f"ob_{ch}", bufs=1) as obp:
        logit = obp.tile([128, 1], f32)
        nc.vector.tensor_reduce(logit[:], t_dots[:],
                                axis=mybir.AxisListType.X, op=alu.add)
        osb = obp.tile([128, 1], f32)
        nc.scalar.activation(osb[:], logit[:],
                             mybir.ActivationFunctionType.Sigmoid,
                             bias=bde[:], scale=1.0)
        nc.sync.dma_start(out_d[ch * BC:(ch + 1) * BC, :], osb[:])


def host_prep(inputs, b_core):
    ids = np.asarray(inputs["ids"])
    tab = np.ascontiguousarray(
        np.asarray(inputs["emb_table"], dtype=np.float32).reshape(F * VOCAB, D))
    w1 = np.asarray(inputs["w1"], np.float32)[0]
    w2 = np.asarray(inputs["w2"], np.float32)[0]
    w3 = np.asarray(inputs["w3"], np.float32)[0]
    k1m = conv_matrix(w1, 23, 6, 1, 4, 28)
    k2m = conv_matrix_cmajor_rows(w2, 20, 5, 4, 4, 24)
    k3m = conv_matrix_cmajor_rows(w3, 7, 3, 4, 4, 9)
    b1r = np.repeat(np.asarray(inputs["b1"], np.float32), 28)[None, :]
    b2r = np.repeat(np.asarray(inputs["b2"], np.float32), 24)[None, :]
    b3r = np.repeat(np.asarray(inputs["b3"], np.float32), 9)[None, :]
    k1e = np.concatenate([k1m, b1r], axis=0)                    # [24, 112]
    k2e = np.concatenate([k2m, b2r], axis=0)                    # [81, 96]
    k3e = np.concatenate([k3m, b3r], axis=0)                    # [29, 36]
    wd = np.asarray(inputs["wd"], np.float32).reshape(D, 3, 4)
    # column block a = h: weight depends on (a, t, c), same for all partitions
    wdtx = np.broadcast_to(
        wd.transpose(1, 0, 2).reshape(1, 3, 256), (128, 3, 256)).copy()
    bde = np.full((128, 1), np.asarray(inputs["bd"], np.float32).ravel()[0],
                  np.float32)
    ident = np.eye(128, dtype=np.float32)
    ioneg = np.broadcast_to(
        (-ETA * (np.arange(28, dtype=np.float64) + 1)).astype(np.float32),
        (128, 28)).copy()
    ones2 = np.zeros((128, 2), np.float32)
    ones2[np.arange(128), np.arange(128) // 64] = 1.0

    nch = b_core // BC
    in_maps = []
    for c in range(NCORES):
        idsc = ids[c * b_core:(c + 1) * b_core].astype(np.int64)
        idc = idsc.reshape(nch, BC, F)
        # gidx[ch, b, f] = f*VOCAB + id
        gidx = (idc + np.arange(F, dtype=np.int64)[None, None, :] * VOCAB
                ).astype(np.int32)
        in_maps.append({
            "tab": tab, "gidx": gidx, "k1e": k1e, "k2e": k2e, "k3e": k3e,
            "wdtx": wdtx, "bde": bde, "ident": ident, "ioneg": ioneg,
            "ones2": ones2, "onesrow": np.ones((1, BC * D), np.float32),
        })
    return in_maps


def kernel(**inputs):
    b_core = np.asarray(inputs["ids"]).shape[0] // NCORES
    nc = build_program(b_core)
    in_maps = host_prep(inputs, b_core)
    res = run_bass_kernel_spmd(nc, in_maps, list(range(NCORES)))
    outs = [np.asarray(r["out"]).reshape(b_core, 1) for r in res.results]
    return np.concatenate(outs, axis=0).astype(np.float32)


# revision 15
# speedup vs baseline: 1.1127x; 1.1127x over previous
"""Trainium2 Bass kernel v2 for nn_CCPM (embedding gather + 3x[conv1d ->
k-max-pool -> relu] + dense + sigmoid), data-parallel over batch on 8 cores.

Per core (B_core = B/8, chunks of BC=128 samples):
- embedding gather: ONE hardware-DGE indirect DMA per chunk (int32 global
  row ids) straight into conv column layout [F, BC*D].
- convs: direct-row matmuls: lhsT = x column-block [K+1, 128] (row 0 = ones),
  rhs = conv matrix with bias row -> PSUM [128cols, C*W] already in row
  layout (no separate transpose for the conv output).
- per-(col,c) top-k: iterated min/max extraction for the k-th order
  statistic, unsegmented prefix-scan + segment-base subtraction for
  compaction shifts, masked shifted adds for in-order compaction.
  All elementwise work is split across DVE (vector) and Pool (gpsimd).
- layer-3 selection absorbed into the dense layer (slot masks); cross-
  partition sample reduction via a tiny PE matmul; sigmoid on Act.
"""
import sys

for p in ("/opt/trn_rl_repo", "/root/.axon_site/_ro/trn_rl_repo"):
    if p not in sys.path:
        sys.path.append(p)

import numpy as np

import concourse.bacc as bacc
import concourse.bass as bass
import concourse.mybir as mybir
import concourse.tile as tile
from concourse.bass import IndirectOffsetOnAxis
from concourse.bass_utils import run_bass_kernel_spmd

F, VOCAB, D = 23, 100000, 64
B_FULL = 16384
NCORES = 8
B_CORE = B_FULL // NCORES
BC = 128
NBLK = BC * D // 128          # 64 column blocks per chunk
ETA = 2.0 ** -96
BIG = 1e30
f32 = mybir.dt.float32
alu = mybir.AluOpType

# NBLK split point: blocks [0, A) on DVE, [A, NBLK) on Pool
SPLIT = {1: 36, 2: 36, 3: 36}


def conv_matrix(w, W_in, kw, C_in, C_out, W_out):
    """K[(w_in, c_in) w-major rows, (c_out, w_out) c-major cols]."""
    K = np.zeros((W_in * C_in, C_out * W_out), np.float32)
    for i in range(W_out):
        for j in range(kw):
            wsrc = i + j - (kw - 1)
            if 0 <= wsrc < W_in:
                for ci in range(C_in):
                    for co in range(C_out):
                        K[wsrc * C_in + ci, co * W_out + i] = w[j, ci, co]
    return K


def conv_matrix_cmajor_rows(w, W_in, kw, C_in, C_out, W_out):
    K = conv_matrix(w, W_in, kw, C_in, C_out, W_out)
    K = K.reshape(W_in, C_in, C_out * W_out).transpose(1, 0, 2)
    return np.ascontiguousarray(K.reshape(C_in * W_in, C_out * W_out))


def bcast_w(ap_3, W):
    """[128, a, c] AP -> [128, a, c, W] with stride-0 W axis."""
    return bass.AP(ap_3.tensor, ap_3.offset,
                   [list(p) for p in ap_3.ap] + [[0, W]])


def bcast_mid(ap_2, n, m):
    """[128, W] AP -> [128, n, m, W] broadcast over middle axes."""
    a = ap_2.ap
    return bass.AP(ap_2.tensor, ap_2.offset,
                   [list(a[0]), [0, n], [0, m], list(a[1])])


def engines(nc, layer):
    return [(nc.vector, 0, NBLK)]


def csplit(layer):
    """compaction block split: DVE [0, A), Pool [A, NBLK)."""
    return SPLIT[layer]


def build_program(b_core):
    nchunks = b_core // BC
    nc = bacc.Bacc(None, target_bir_lowering=False, debug=False)

    tab_d = nc.dram_tensor("tab", [F * VOCAB, D], f32, kind="ExternalInput")
    gidx_d = nc.dram_tensor("gidx", [nchunks, 128, F], mybir.dt.int32,
                            kind="ExternalInput")
    k1_d = nc.dram_tensor("k1e", [24, 112], f32, kind="ExternalInput")
    k2_d = nc.dram_tensor("k2e", [81, 96], f32, kind="ExternalInput")
    k3_d = nc.dram_tensor("k3e", [29, 36], f32, kind="ExternalInput")
    wdt_d = nc.dram_tensor("wdtx", [128, 3, 256], f32, kind="ExternalInput")
    bd_d = nc.dram_tensor("bde", [128, 1], f32, kind="ExternalInput")
    id_d = nc.dram_tensor("ident", [128, 128], f32, kind="ExternalInput")
    ineg_d = nc.dram_tensor("ioneg", [128, 28], f32, kind="ExternalInput")
    on2_d = nc.dram_tensor("ones2", [128, 2], f32, kind="ExternalInput")
    onr_d = nc.dram_tensor("onesrow", [1, BC * D], f32, kind="ExternalInput")
    import os as _os
    dbg = _os.environ.get("V2DBG") == "1"
    if dbg:
        zd1 = nc.dram_tensor("zdbg1", [128, NBLK, 80], f32, kind="ExternalOutput")
        zd2 = nc.dram_tensor("zdbg2", [128, NBLK, 28], f32, kind="ExternalOutput")
        rd1 = nc.dram_tensor("rdbg1", [128, NBLK, 112], f32, kind="ExternalOutput")
        dd = nc.dram_tensor("ddbg", [128, NBLK], f32, kind="ExternalOutput")
        wsd = nc.dram_tensor("wsdbg", [128, NBLK, 36], f32, kind="ExternalOutput")
        qd = nc.dram_tensor("qdbg", [128, NBLK, 36], f32, kind="ExternalOutput")
        phd = nc.dram_tensor("phdbg", [128, NBLK, 36], f32, kind="ExternalOutput")
        rd3 = nc.dram_tensor("rdbg3", [128, NBLK, 36], f32, kind="ExternalOutput")
    else:
        zd1 = zd2 = rd1 = dd = wsd = qd = phd = rd3 = None
    out_d = nc.dram_tensor("out", [b_core, 1], f32, kind="ExternalOutput")

    with tile.TileContext(nc) as tc:
        with tc.tile_pool(name="glob", bufs=1) as gp:
            k1e = gp.tile([24, 112], f32)
            nc.sync.dma_start(k1e[:], k1_d[:])
            k2e = gp.tile([81, 96], f32)
            nc.sync.dma_start(k2e[:], k2_d[:])
            k3e_t = gp.tile([93, 36], f32)
            nc.sync.dma_start(k3e_t[64:93], k3_d[:])
            k3e = k3e_t[64:93]
            wdt = gp.tile([128, 3, 256], f32)
            nc.sync.dma_start(wdt[:], wdt_d[:])
            bde = gp.tile([128, 1], f32)
            nc.sync.dma_start(bde[:], bd_d[:])
            ident = gp.tile([128, 128], f32)
            nc.sync.dma_start(ident[:], id_d[:])
            ioneg = gp.tile([128, 28], f32)
            nc.sync.dma_start(ioneg[:], ineg_d[:])
            ones2 = gp.tile([128, 2], f32)
            nc.sync.dma_start(ones2[:], on2_d[:])

            # big static arenas (shared across chunks)
            xcol = gp.tile([24, BC * D], f32)      # L1 gather dst, row 0=ones
            zcol = gp.tile([93, BC * D], f32)      # L2/L3 conv input, row 0=ones
            t_r = gp.tile([128, NBLK, 112], f32)   # conv outputs (row layout)
            t_w = gp.tile([128, NBLK, 112], f32)   # work / incl / shift-tilde
            t_e = gp.tile([128, NBLK, 112], f32)   # eqz / drop / wsel
            t_z = gp.tile([128, NBLK, 80], f32)    # compacted z (L1: 80, L2: 28)
            t_t = gp.tile([128, NBLK, 96], f32)    # compact tmp / q3 / scratch
            t_m = gp.tile([128, NBLK, 4], f32)     # per-group reduce result
            t_sp = gp.tile([128, NBLK * 4 + 128], f32)   # segment bases
            t_dots = gp.tile([128, NBLK], f32)
            gidx = gp.tile([128, F], mybir.dt.int32)
            erow = gp.tile([128, F, D], f32)

            nc.sync.dma_start(xcol[23:24, :], onr_d[:])
            nc.sync.dma_start(zcol[80:81, :], onr_d[:])
            nc.sync.dma_start(zcol[92:93, :], onr_d[:])

            for ch in range(nchunks):
                build_chunk(nc, tc, ch, tab_d, gidx_d, onr_d, out_d,
                            k1e, k2e, k3e, wdt, bde, ident, ioneg, ones2,
                            xcol, zcol, t_r, t_w, t_e, t_z, t_t, t_m, t_sp,
                            t_dots, gidx, erow,
                            dbg=(zd1, zd2, rd1, dd, wsd, qd, phd, rd3) if (dbg and ch == 0) else None)
    nc.compile()
    return nc


def packed(t, cw):
    """[128, NBLK, cw] packed (contiguous) view of a flat work tile."""
    return t[:].rearrange("p a m -> p (a m)")[:, 0:NBLK * cw].rearrange(
        "p (a cw) -> p a cw", cw=cw)


def conv_rows(nc, tc, ch, tag, xarena, K, kmat, M, t_r, batch=4):
    """Direct-row conv: out row-block jb = x[:, jb*128:+128].T @ kmat.
    Writes packed(t_r, M)[:, jb, :]."""
    rv = packed(t_r, M)
    with tc.tile_pool(name=f"cps{tag}_{ch}", bufs=2,
                      space=bass.MemorySpace.PSUM) as psum:
        for g in range(NBLK // batch):
            ps = psum.tile([128, batch * M], f32, name=f"ps{tag}",
                           tag=f"ps{tag}")
            for q in range(batch):
                jb = g * batch + q
                nc.tensor.matmul(ps[:, q * M:(q + 1) * M],
                                 xarena[0:K, jb * 128:(jb + 1) * 128],
                                 kmat, start=True, stop=True)
            nc.scalar.activation(
                rv[:, g * batch:(g + 1) * batch, :].rearrange(
                    "p a m -> p (a m)"),
                ps[:], mybir.ActivationFunctionType.Copy, bias=0.0)


def transpose_cols(nc, tc, ch, tag, t_z, zw, zarena, row0, batch=4):
    """z row blocks [128, zw] -> column layout into zarena rows row0..row0+zw."""
    zv = packed(t_z, zw)
    with tc.tile_pool(name=f"tps{tag}_{ch}", bufs=2,
                      space=bass.MemorySpace.PSUM) as psum:
        ident = transpose_cols.ident
        for g in range(NBLK // batch):
            ps = psum.tile([zw, batch * 128], f32, name=f"tp{tag}",
                           tag=f"tp{tag}")
            for q in range(batch):
                jb = g * batch + q
                nc.tensor.transpose(ps[:, q * 128:(q + 1) * 128],
                                    zv[:, jb, :], ident[:])
            nc.scalar.activation(
                zarena[row0:row0 + zw,
                       g * batch * 128:(g + 1) * batch * 128],
                ps[:], mybir.ActivationFunctionType.Copy, bias=0.0)


def ereduce(eng, is_dve, m, wk, W, C, op, scr_flat):
    """Per-group reduce along W into m [128, nb, C]. DVE: native X-reduce;
    Pool: pairwise fold tree (GpSimd has no free-axis tensor_reduce)."""
    if is_dve:
        eng.tensor_reduce(m, wk, axis=mybir.AxisListType.X, op=op)
        return
    w1 = (W + 1) // 2
    w2 = (w1 + 1) // 2
    regA = scr_flat[:, :, 0:C * w1].rearrange("p a (c w) -> p a c w", c=C)
    regB = scr_flat[:, :, C * w1:C * (w1 + w2)].rearrange(
        "p a (c w) -> p a c w", c=C)
    cur, width, use_a = wk, W, True
    while width > 1:
        half, rem = width // 2, width % 2
        dst = regA if use_a else regB
        eng.tensor_tensor(dst[:, :, :, 0:half], cur[:, :, :, 0:half],
                          cur[:, :, :, half:2 * half], op=op)
        if rem:
            eng.tensor_copy(dst[:, :, :, half:half + 1],
                            cur[:, :, :, 2 * half:2 * half + 1])
        width = half + rem
        cur, use_a = dst, not use_a
    eng.tensor_copy(m, cur[:, :, :, 0:1].rearrange("p a c w -> p a (c w)"))


def packed_off(t, off, cw):
    """[128, NBLK, cw] packed view of flat tile starting at free offset."""
    return t[:].rearrange("p a m -> p (a m)")[:, off:off + NBLK * cw].rearrange(
        "p (a cw) -> p a cw", cw=cw)


def bcast_g(ap_2, G):
    """[128, W] AP -> [128, G, W] broadcast over middle axis."""
    a = ap_2.ap
    return bass.AP(ap_2.tensor, ap_2.offset, [list(a[0]), [0, G], list(a[1])])


def bcast_w3(ap_2, W):
    """[128, G] AP -> [128, G, W] with stride-0 W axis."""
    return bass.AP(ap_2.tensor, ap_2.offset,
                   [list(p) for p in ap_2.ap] + [[0, W]])


def g3(pv, a0, a1, C):
    """packed [128, NBLK, C*W] view -> [128, (nb C), W] 3D slice."""
    return pv[:, a0:a1].rearrange("p a (c w) -> p (a c) w", c=C)


def ereduce3(eng, is_dve, m, wk, W, scrA, scrB):
    """reduce along W: wk [128, G, W] -> m [128, G]."""
    if is_dve:
        eng.tensor_reduce(m, wk, axis=mybir.AxisListType.X, op=ereduce3.op)
        return
    op = ereduce3.op
    cur, width, use_a = wk, W, True
    while width > 1:
        half, rem = width // 2, width % 2
        dst = scrA if use_a else scrB
        eng.tensor_tensor(dst[:, :, 0:half], cur[:, :, 0:half],
                          cur[:, :, half:2 * half], op=op)
        if rem:
            eng.tensor_copy(dst[:, :, half:half + 1],
                            cur[:, :, 2 * half:2 * half + 1])
        width = half + rem
        cur, use_a = dst, not use_a
    eng.tensor_copy(m, cur[:, :, 0])


def topk_stage(nc, layer, W, C, k, largest, t_r, t_w, t_e, t_z, t_t, t_m,
               t_sp, ioneg, uniquify):
    """Full top-k + compaction for one layer, split across DVE and Pool.
    All stt ops use 3D [128, G=(nb*C), W] views (walrus requires <=3D)."""
    nrounds = k if largest else W - k
    CW = C * W
    rv, wv, ev = packed(t_r, CW), packed(t_w, CW), packed(t_e, CW)
    zv = packed(t_z, C * k)
    mv = packed(t_m, 4)
    w1 = (W + 1) // 2
    w2 = (w1 + 1) // 2
    sA = packed_off(t_t, 0, C * w1)
    sB = packed_off(t_t, NBLK * C * w1, C * w2)
    for eng, a0, a1 in engines(nc, layer):
        nb = a1 - a0
        r = g3(rv, a0, a1, C)
        wk = g3(wv, a0, a1, C)
        if uniquify:
            eng.scalar_tensor_tensor(wk, r, 0.0,
                                     bcast_g(ioneg[:, 0:W], nb * C),
                                     op0=alu.is_equal, op1=alu.mult)
            eng.tensor_tensor(r, r, wk, op=alu.add)

    # extraction works on a copy; t_r stays pristine for drop + compaction
    nc.scalar.copy(wv.rearrange("p a cw -> p (a cw)"),
                   rv.rearrange("p a cw -> p (a cw)"))

    red_op = alu.max if largest else alu.min
    kill = -BIG if largest else BIG
    for ei, (eng, a0, a1) in enumerate(engines(nc, layer)):
        nb = a1 - a0
        G = nb * C
        r = g3(rv, a0, a1, C)
        wk = g3(wv, a0, a1, C)
        eq = g3(ev, a0, a1, C)
        m = mv[:, a0:a1].rearrange("p a c -> p (a c)")
        mb = bcast_w3(m, W)
        ereduce3.op = red_op
        for t in range(nrounds):
            ereduce3(eng, ei == 0, m, wk, W,
                     g3(sA, a0, a1, C), g3(sB, a0, a1, C))
            if t < nrounds - 1:
                eng.tensor_tensor(eq, wk, mb, op=alu.is_equal)
                eng.scalar_tensor_tensor(wk, eq, kill, wk,
                                         op0=alu.mult, op1=alu.add)
        # drop mask vs tau (= last extracted value)
        dop = alu.is_lt if largest else alu.is_le
        eng.tensor_tensor(eq, r, mb, op=dop)
        # compaction shifts: unsegmented cumsum + segment-base subtract
        eng.tensor_tensor_scan(
            wv[:, a0:a1].rearrange("p a cw -> p (a cw)"),
            ev[:, a0:a1].rearrange("p a cw -> p (a cw)"),
            ev[:, a0:a1].rearrange("p a cw -> p (a cw)"),
            0.0, op0=alu.add, op1=alu.bypass)
        incl = wk
        sp2 = t_sp[:, a0 * C:a1 * C]
        eng.memset(sp2[:, 0:1], 0.0)
        eng.tensor_copy(sp2[:, 1:G], incl[:, 0:G - 1, W - 1])
        eng.tensor_tensor(incl, incl, bcast_w3(sp2, W), op=alu.subtract)
        # shift-tilde in place: st = 99*drop + (incl - segbase)
        eng.scalar_tensor_tensor(incl, eq, 99.0, incl,
                                 op0=alu.mult, op1=alu.add)
    # in-order compaction: z[t] = r[t+s] where st[t+s] == s
    # DVE blocks [0, A): fused stt form; Pool blocks [A, NBLK): ts+tt form
    A = csplit(layer)
    stv, srv = wv, rv
    ttv = packed_off(t_t, 0, C * k)
    mkv = packed_off(t_e, 0, C * k)   # pool mask scratch (drop mask is dead)
    for s in (range(W - k + 1) if A > 0 else ()):
        st3 = g3(stv, 0, A, C)
        sr3 = g3(srv, 0, A, C)
        z = g3(zv, 0, A, C)
        tt = g3(ttv, 0, A, C)
        if s == 0:
            nc.vector.scalar_tensor_tensor(z, st3[:, :, 0:k], 0.0,
                                           sr3[:, :, 0:k],
                                           op0=alu.is_equal, op1=alu.mult)
        else:
            nc.vector.scalar_tensor_tensor(tt, st3[:, :, s:s + k], float(s),
                                           sr3[:, :, s:s + k],
                                           op0=alu.is_equal, op1=alu.mult)
            nc.vector.tensor_tensor(z, z, tt, op=alu.add)
    for s in (range(W - k + 1) if A < NBLK else ()):
        st3 = g3(stv, A, NBLK, C)
        sr3 = g3(srv, A, NBLK, C)
        z = g3(zv, A, NBLK, C)
        tt = g3(ttv, A, NBLK, C)
        mk = g3(mkv, A, NBLK, C)
        nc.gpsimd.tensor_scalar(mk, st3[:, :, s:s + k], float(s), None,
                                op0=alu.is_equal)
        if s == 0:
            nc.gpsimd.tensor_tensor(z, mk, sr3[:, :, 0:k], op=alu.mult)
        else:
            nc.gpsimd.tensor_tensor(tt, mk, sr3[:, :, s:s + k], op=alu.mult)
            nc.gpsimd.tensor_tensor(z, z, tt, op=alu.add)


def build_chunk(nc, tc, ch, tab_d, gidx_d, onr_d, out_d, k1e, k2e, k3e,
                wdt, bde, ident, ioneg, ones2, xcol, zcol, t_r, t_w, t_e,
                t_z, t_t, t_m, t_sp, t_dots, gidx, erow, dbg=None):
    transpose_cols.ident = ident

    # ---------------- gather (per-feature, one offset per partition) ------
    nc.sync.dma_start(gidx[:], gidx_d[ch])
    for f in range(F):
        nc.gpsimd.indirect_dma_start(
            out=erow[:, f, :],
            out_offset=None,
            in_=tab_d[:],
            in_offset=IndirectOffsetOnAxis(ap=gidx[:, f:f + 1], axis=0))
    # transpose to column layout xcol[f, (h b)]
    with tc.tile_pool(name=f"eps_{ch}", bufs=2,
                      space=bass.MemorySpace.PSUM) as epsum:
        xv = xcol[0:F, :].rearrange("f (h b) -> f h b", h=D)
        for g in range(D // 4):
            pe = epsum.tile([F, 4 * 128], f32, name="pte", tag="pte")
            for q in range(4):
                h = g * 4 + q
                nc.tensor.transpose(pe[:, q * 128:(q + 1) * 128],
                                    erow[:, :, h], ident[:])
            nc.scalar.activation(
                xv[:, g * 4:(g + 1) * 4, :].rearrange("f h b -> f (h b)"),
                pe[:], mybir.ActivationFunctionType.Copy, bias=0.0)

    # ---------------- layer 1 ----------------
    conv_rows(nc, tc, ch, "1", xcol, 24, k1e[:], 112, t_r, batch=4)
    topk_stage(nc, 1, 28, 4, 20, False, t_r, t_w, t_e, t_z, t_t, t_m,
               t_sp, ioneg, uniquify=False)
    if dbg is not None:
        nc.sync.dma_start(dbg[2][:], packed(t_r, 112))
    zv1 = packed(t_z, 80).rearrange("p a cw -> p (a cw)")
    nc.scalar.activation(zv1, zv1, mybir.ActivationFunctionType.Relu)
    if dbg is not None:
        nc.sync.dma_start(dbg[0][:], packed(t_z, 80))
    transpose_cols(nc, tc, ch, "1", t_z, 80, zcol, 0, batch=4)

    # ---------------- layer 2 ----------------
    conv_rows(nc, tc, ch, "2", zcol, 81, k2e[:], 96, t_r, batch=4)
    topk_stage(nc, 2, 24, 4, 7, True, t_r, t_w, t_e, t_z, t_t, t_m,
               t_sp, ioneg, uniquify=True)
    zv2 = packed(t_z, 28).rearrange("p a cw -> p (a cw)")
    nc.scalar.activation(zv2, zv2, mybir.ActivationFunctionType.Relu)
    if dbg is not None:
        nc.sync.dma_start(dbg[1][:], packed(t_z, 28))
    transpose_cols(nc, tc, ch, "2", t_z, 28, zcol, 64, batch=4)

    # ---------------- layer 3 + dense ----------------
    conv_rows(nc, tc, ch, "3", zcol[64:93], 29, k3e, 36, t_r, batch=8)
    # L3 transposes clobbered L2's ones row (80); restore AFTER conv3 read it
    nc.sync.dma_start(zcol[80:81, :], onr_d[:])
    if dbg is not None:
        nc.sync.dma_start(dbg[7][:], packed(t_r, 36))
    W, C, k = 9, 4, 3
    rv, wv, ev = packed(t_r, 36), packed(t_w, 36), packed(t_e, 36)
    qv = packed(t_t, 36)
    sv = packed(t_z, 36)  # scratch (z2k already consumed)
    mv = packed(t_m, 4)
    sA = packed_off(t_z, NBLK * 36, 4 * 5)   # pool reduce scratch in t_z tail
    sB = packed_off(t_z, NBLK * 36 + NBLK * 20, 4 * 3)
    for eng, a0, a1 in engines(nc, 3):
        nb = a1 - a0
        r = g3(rv, a0, a1, C)
        wk = g3(wv, a0, a1, C)
        eng.scalar_tensor_tensor(wk, r, 0.0, bcast_g(ioneg[:, 0:W], nb * C),
                                 op0=alu.is_equal, op1=alu.mult)
        eng.tensor_tensor(r, r, wk, op=alu.add)
    nc.scalar.copy(wv.rearrange("p a cw -> p (a cw)"),
                   rv.rearrange("p a cw -> p (a cw)"))
    for ei, (eng, a0, a1) in enumerate(engines(nc, 3)):
        nb = a1 - a0
        G = nb * C
        r = g3(rv, a0, a1, C)
        wk = g3(wv, a0, a1, C)
        eq = g3(ev, a0, a1, C)
        m = mv[:, a0:a1].rearrange("p a c -> p (a c)")
        mb = bcast_w3(m, W)
        ereduce3.op = alu.max
        for t in range(k):
            ereduce3(eng, ei == 0, m, wk, W,
                     g3(sA, a0, a1, C), g3(sB, a0, a1, C))
            if t < k - 1:
                eng.tensor_tensor(eq, wk, mb, op=alu.is_equal)
                eng.scalar_tensor_tensor(wk, eq, -BIG, wk,
                                         op0=alu.mult, op1=alu.add)
        eng.tensor_tensor(eq, r, mb, op=alu.is_ge)  # sel
        eng.tensor_tensor_scan(
            wv[:, a0:a1].rearrange("p a cw -> p (a cw)"),
            ev[:, a0:a1].rearrange("p a cw -> p (a cw)"),
            ev[:, a0:a1].rearrange("p a cw -> p (a cw)"),
            0.0, op0=alu.add, op1=alu.bypass)
        incl = wk
        sp2 = t_sp[:, a0 * C:a1 * C]
        eng.memset(sp2[:, 0:1], 0.0)
        eng.tensor_copy(sp2[:, 1:G], incl[:, 0:G - 1, W - 1])
        eng.tensor_tensor(incl, incl, bcast_w3(sp2, W), op=alu.subtract)
        # phat = incl - 101*sel: selected -> {-100,-99,-98}, dropped >= 0
        eng.scalar_tensor_tensor(incl, eq, -101.0, incl,
                                 op0=alu.mult, op1=alu.add)
    # q3 = relu(r3) on Act
    nc.scalar.activation(qv.rearrange("p a cw -> p (a cw)"),
                         rv.rearrange("p a cw -> p (a cw)"),
                         mybir.ActivationFunctionType.Relu)
    # wsel/dots: DVE blocks [0, A3), Pool blocks [A3, NBLK) in legal forms
    A3 = 40
    for part, (a0, a1) in enumerate(((0, A3), (A3, NBLK))):
        nb = a1 - a0
        G = nb * C
        phat = g3(wv, a0, a1, C)
        wsel = g3(ev, a0, a1, C)
        scr = g3(sv, a0, a1, C)
        q3 = g3(qv, a0, a1, C)
        wtbs = []
        for t in range(k):
            wtbs.append(bass.AP(
                wdt.tensor, wdt[:, t, a0 * C:a1 * C].offset,
                [list(wdt[:].ap[0]), [1, G], [0, W]]))
        if part == 0:
            for t in range(k):
                if t == 0:
                    nc.vector.scalar_tensor_tensor(
                        wsel, phat, float(t - 100), wtbs[t],
                        op0=alu.is_equal, op1=alu.mult)
                else:
                    nc.vector.scalar_tensor_tensor(
                        scr, phat, float(t - 100), wtbs[t],
                        op0=alu.is_equal, op1=alu.mult)
                    nc.vector.tensor_tensor(wsel, wsel, scr, op=alu.add)
            nc.vector.tensor_tensor(q3, q3, wsel, op=alu.mult)
            m = mv[:, a0:a1].rearrange("p a c -> p (a c)")
            nc.vector.tensor_reduce(m, q3, axis=mybir.AxisListType.X,
                                    op=alu.add)
            m4 = mv[:, a0:a1]
            s2 = t_sp[:, 256 + a0 * 2:256 + a0 * 2 + nb * 2].rearrange(
                "p (a c) -> p a c", c=2)
            nc.vector.tensor_tensor(s2, m4[:, :, 0:2], m4[:, :, 2:4],
                                    op=alu.add)
            nc.vector.tensor_tensor(t_dots[:, a0:a1], s2[:, :, 0],
                                    s2[:, :, 1], op=alu.add)
        else:
            for t in range(k):
                if t == 0:
                    nc.gpsimd.tensor_scalar(wsel, phat, float(t - 100), None,
                                            op0=alu.is_equal)
                    nc.gpsimd.tensor_tensor(wsel, wsel, wtbs[t], op=alu.mult)
                else:
                    nc.gpsimd.tensor_scalar(scr, phat, float(t - 100), None,
                                            op0=alu.is_equal)
                    nc.gpsimd.tensor_tensor(scr, scr, wtbs[t], op=alu.mult)
                    nc.gpsimd.tensor_tensor(wsel, wsel, scr, op=alu.add)
            nc.gpsimd.tensor_tensor(q3, q3, wsel, op=alu.mult)
            # add-fold tree (pool-legal): 36 -> 1 per group via halving on W
            m = mv[:, a0:a1].rearrange("p a c -> p (a c)")
            ereduce3.op = alu.add
            ereduce3(nc.gpsimd, False, m, q3, W,
                     g3(sA, a0, a1, C), g3(sB, a0, a1, C))
            m4 = mv[:, a0:a1]
            s2 = t_sp[:, 256 + a0 * 2:256 + a0 * 2 + nb * 2].rearrange(
                "p (a c) -> p a c", c=2)
            nc.gpsimd.tensor_tensor(s2, m4[:, :, 0:2], m4[:, :, 2:4],
                                    op=alu.add)
            nc.gpsimd.tensor_tensor(t_dots[:, a0:a1], s2[:, :, 0],
                                    s2[:, :, 1], op=alu.add)
    # per-sample logit: partition p = b -> reduce over h (free axis)
    with tc.tile_pool(name=f"ob_{ch}", bufs=1) as obp:
        logit = obp.tile([128, 1], f32)
        nc.vector.tensor_reduce(logit[:], t_dots[:],
                                axis=mybir.AxisListType.X, op=alu.add)
        osb = obp.tile([128, 1], f32)
        nc.scalar.activation(osb[:], logit[:],
                             mybir.ActivationFunctionType.Sigmoid,
                             bias=bde[:], scale=1.0)
        nc.sync.dma_start(out_d[ch * BC:(ch + 1) * BC, :], osb[:])


def host_prep(inputs, b_core):
    ids = np.asarray(inputs["ids"])
    tab = np.ascontiguousarray(
        np.asarray(inputs["emb_table"], dtype=np.float32).reshape(F * VOCAB, D))
    w1 = np.asarray(inputs["w1"], np.float32)[0]
    w2 = np.asarray(inputs["w2"], np.float32)[0]
    w3 = np.asarray(inputs["w3"], np.float32)[0]
    k1m = conv_matrix(w1, 23, 6, 1, 4, 28)
    k2m = conv_matrix_cmajor_rows(w2, 20, 5, 4, 4, 24)
    k3m = conv_matrix_cmajor_rows(w3, 7, 3, 4, 4, 9)
    b1r = np.repeat(np.asarray(inputs["b1"], np.float32), 28)[None, :]
    b2r = np.repeat(np.asarray(inputs["b2"], np.float32), 24)[None, :]
    b3r = np.repeat(np.asarray(inputs["b3"], np.float32), 9)[None, :]
    k1e = np.concatenate([k1m, b1r], axis=0)                    # [24, 112]
    k2e = np.concatenate([k2m, b2r], axis=0)                    # [81, 96]
    k3e = np.concatenate([k3m, b3r], axis=0)                    # [29, 36]
    wd = np.asarray(inputs["wd"], np.float32).reshape(D, 3, 4)
    # column block a = h: weight depends on (a, t, c), same for all partitions
    wdtx = np.broadcast_to(
        wd.transpose(1, 0, 2).reshape(1, 3, 256), (128, 3, 256)).copy()
    bde = np.full((128, 1), np.asarray(inputs["bd"], np.float32).ravel()[0],
                  np.float32)
    ident = np.eye(128, dtype=np.float32)
    ioneg = np.broadcast_to(
        (-ETA * (np.arange(28, dtype=np.float64) + 1)).astype(np.float32),
        (128, 28)).copy()
    ones2 = np.zeros((128, 2), np.float32)
    ones2[np.arange(128), np.arange(128) // 64] = 1.0

    nch = b_core // BC
    in_maps = []
    for c in range(NCORES):
        idsc = ids[c * b_core:(c + 1) * b_core].astype(np.int64)
        idc = idsc.reshape(nch, BC, F)
        # gidx[ch, b, f] = f*VOCAB + id
        gidx = (idc + np.arange(F, dtype=np.int64)[None, None, :] * VOCAB
                ).astype(np.int32)
        in_maps.append({
            "tab": tab, "gidx": gidx, "k1e": k1e, "k2e": k2e, "k3e": k3e,
            "wdtx": wdtx, "bde": bde, "ident": ident, "ioneg": ioneg,
            "ones2": ones2, "onesrow": np.ones((1, BC * D), np.float32),
        })
    return in_maps


def kernel(**inputs):
    b_core = np.asarray(inputs["ids"]).shape[0] // NCORES
    nc = build_program(b_core)
    in_maps = host_prep(inputs, b_core)
    res = run_bass_kernel_spmd(nc, in_maps, list(range(NCORES)))
    outs = [np.asarray(r["out"]).reshape(b_core, 1) for r in res.results]
    return np.concatenate(outs, axis=0).astype(np.float32)
